# revision 34
# baseline (speedup 1.0000x reference)
"""MultiHeadSemGConv Trainium2 kernel.

Computes, for x:[B,N,CIN], W:[H,2,CIN,HC], e:[H,N*K], bias:[H,HC],
rows/cols:[N*K] (int32 edge list):

    h = einsum('bnc,hscd->shbnd', x, W)             # two projections per head
    A = softmax(scatter(e at (rows,cols), NEG))     # [H,N,N]
    out[h,b] = diag(A)*h0 + (A - diag)@h1 + bias    # -> [B,N,H*HC]

Strategy: pure data-parallel over batch across 8 NeuronCores.  The tiny
[H,98,98] adjacency softmax is precomputed on host.  Per core:

  - host ships x pre-transposed and pre-cast: xT [256, 12576] fp16
    (halves input HBM traffic; removes all on-device transposes)
  - phase 1, per sample b: h[128,512] = xT[:, 98b:98b+128].T @ Wall
    (2 accumulating fp16 matmuls, f32 PSUM), 1 sample per PSUM bank
  - PSUM -> SBUF fp16 copies split across Scalar/Vector engines into a
    per-group [128, 8, 512] tile whose row 98 holds the bias (loaded
    once; an all-ones row 98 in the graph matrix adds it in phase 2)
  - phase 2, per 8-sample group, per head: ONE off-diagonal matmul
    (A_off^T zero-padded to 128 + bias ones-row); the diagonal term is
    fused into the merge on DVE via scalar_tensor_tensor:
    osb = h0*dg + (A_off@h1 + bias)  -> fp16, DMA out, host upcasts.
    The last two groups fold the diag back into matmuls (PE is idle at
    the tail) so the merge is a plain parallel copy.
"""

import os
import sys

import numpy as np

try:
    import concourse.bass as bass  # noqa: F401
except Exception:  # pragma: no cover - fresh grading dir fallback
    for p in ("/opt/trn_rl_repo", "/root/.axon_site/_ro/trn_rl_repo"):
        if os.path.isdir(p) and p not in sys.path:
            sys.path.insert(0, p)
    import concourse.bass as bass  # noqa: F401

# ---------------------------------------------------------------- constants
NLM = 98          # landmarks (graph nodes)
HEADS = 4
CIN = 256
HC = 64
HD = 512          # h width = 2 (s) * 4 (heads) * 64 (d)
B = 1024
NCORES = 8
NS = B // NCORES  # samples per core = 128
P = 128
G = 8             # samples per output group
NGRP = NS // G    # 16 groups per core
GW = G * NLM      # xT cols per group = 784
WPAD = NS * NLM + 32   # 12576: xT padded so the last 128-wide window fits
NEG = -9e15

_CACHE = {}


def _build_nc():
    import concourse.mybir as mybir
    import concourse.tile as tile
    from concourse import bacc

    f16 = mybir.dt.float16
    f32 = mybir.dt.float32

    nc = bacc.Bacc(None, target_bir_lowering=False)

    xt_d = nc.dram_tensor("xt", [2 * P, WPAD], f16, kind="ExternalInput")
    wall = nc.dram_tensor("wall", [P, 2, HD], f16, kind="ExternalInput")
    # gm | gmd | dgv packed in one blob to keep the head DMA queue short
    gblob_d = nc.dram_tensor(
        "gblob", [P, 2 * HEADS * P + HEADS], f16, kind="ExternalInput"
    )
    brow = nc.dram_tensor("brow", [30, G * HD], f16, kind="ExternalInput")
    out = nc.dram_tensor("out", [NS * NLM, CIN], f16, kind="ExternalOutput")

    xt_src = xt_d[:].rearrange("(c p) w -> p c w", p=P)

    with tile.TileContext(nc) as tc:
        with (
            tc.tile_pool(name="const", bufs=1) as constp,
            tc.tile_pool(name="xt", bufs=1) as xtp,
            tc.tile_pool(name="hgrp", bufs=1) as hgp,
            tc.tile_pool(name="osb", bufs=3) as osbp,
            tc.tile_pool(name="phs", bufs=4, space="PSUM") as phsp,
            tc.tile_pool(name="po3", bufs=2, space="PSUM") as po3p,
        ):
            xt_sb = xtp.tile([P, 2, WPAD], f16, tag="xt")

            def emit_in(c0, c1):
                nc.sync.dma_start(
                    xt_sb[:, :, c0:c1], xt_src[:, :, c0:c1]
                )

            def emit_chunk(c):
                # chunk c covers groups 2c, 2c+1
                emit_in(2 * c * GW, WPAD if c == 7 else 2 * (c + 1) * GW)

            # HAM warm-up: dependency-free dummy matmuls on scratch SBUF
            # run while the first input DMAs land, so real matmuls start
            # at full clock instead of K=4/8 half rate
            scratch = constp.tile([P, P], f16, tag="scratch")
            nc.gpsimd.memset(scratch[:], 0.0)
            for _ in range(12):
                wps = phsp.tile([P, HD], f32, tag="hps")
                nc.tensor.matmul(
                    wps[:, 0:P], scratch[:], scratch[:], start=True, stop=True
                )
            # wall + first two samples land first so the PE starts early
            wall_sb = constp.tile([P, 2, HD], f16, tag="wall")
            nc.scalar.dma_start(wall_sb[:], wall[:])
            emit_in(0, 2 * NLM + 30)
            emit_in(2 * NLM + 30, 7 * NLM + P)
            emit_in(7 * NLM + P, 2 * GW)
            emit_chunk(1)
            gblob = constp.tile([P, 2 * HEADS * P + HEADS], f16, tag="gblob")
            nc.sync.dma_start(gblob[:], gblob_d[:])
            gm_sb = gblob[:, 0 : HEADS * P].rearrange("p (h i) -> p h i", h=HEADS)
            gmd_sb = gblob[:, HEADS * P : 2 * HEADS * P].rearrange(
                "p (h i) -> p h i", h=HEADS
            )
            dg_sb = gblob[0:NLM, 2 * HEADS * P : 2 * HEADS * P + HEADS]
            emit_chunk(2)

            # group buffers; rows 98..127 initialized once (row 98 =
            # bias pattern used by the ones-row in gm, 99..127 zeros)
            NHG = 4
            hgrp = [
                hgp.tile([P, G, HD], f16, tag=f"hgrp{k}", name=f"hgrp{k}")
                for k in range(NHG)
            ]
            for k in range(NHG):
                nc.sync.dma_start(
                    hgrp[k][NLM:P],
                    brow[:].rearrange("r (s f) -> r s f", s=G),
                )

            for gi in range(NGRP):
                if gi % 2 == 0 and gi // 2 + 3 <= 7:
                    emit_chunk(gi // 2 + 3)
                hg = hgrp[gi % NHG]
                # the last two groups fold the diag term back into
                # matmuls (PE is idle at the tail) so the merge is a
                # plain parallel copy instead of the serial DVE STT chain
                last2 = gi >= NGRP - 2
                ndve = (4 if gi == NGRP - 1 else 2) if last2 else 2
                # -------- phase 1: project 8 samples, 1 per PSUM tile
                for si in range(G):
                    b = gi * G + si
                    hps = phsp.tile([P, HD], f32, tag="hps")
                    for cc in range(2):
                        nc.tensor.matmul(
                            hps[:],
                            xt_sb[:, cc, NLM * b : NLM * b + P],
                            wall_sb[:, cc, :],
                            start=(cc == 0),
                            stop=(cc == 1),
                        )
                    if si < ndve:
                        nc.vector.tensor_copy(
                            hg[0:NLM, si, :], hps[0:NLM]
                        )
                    else:
                        nc.scalar.copy(
                            out=hg[0:NLM, si, :], in_=hps[0:NLM]
                        )
                # -------- phase 2: off-diag graph mix per head (bias via
                # the gm ones-row against hgrp row 98); then merge the
                # diag term on DVE: osb = h0*dg + (A_off@h1 + bias)
                osb = osbp.tile([NLM, G, CIN], f16, tag="osb")
                ov_h = osb[:].rearrange("i s (h d) -> i h s d", h=HEADS)
                for hp in range(2):
                    po3 = po3p.tile([P, 2, HD], f32, tag="po3")
                    po_h = po3[:].rearrange("i h (s d) -> i h s d", s=G)
                    for k in range(2):
                        hd = 2 * hp + k
                        nc.tensor.matmul(
                            po_h[:, k],
                            gm_sb[:, hd, :],
                            hg[:, :, 256 + hd * HC : 256 + (hd + 1) * HC],
                            start=True,
                            stop=not last2,
                        )
                        if last2:
                            nc.tensor.matmul(
                                po_h[:, k],
                                gmd_sb[:, hd, :],
                                hg[:, :, hd * HC : (hd + 1) * HC],
                                start=False,
                                stop=True,
                            )
                    if last2:
                        if gi == NGRP - 1:
                            # split by sample halves so the first store
                            # overlaps the second merge
                            for sh in range(2):
                                s0, s1 = sh * (G // 2), (sh + 1) * (G // 2)
                                if hp == 0:
                                    nc.scalar.copy(
                                        out=ov_h[:, 0:2, s0:s1],
                                        in_=po_h[0:NLM, :, s0:s1],
                                    )
                                else:
                                    nc.vector.tensor_copy(
                                        ov_h[:, 2:4, s0:s1],
                                        po_h[0:NLM, :, s0:s1],
                                    )
                        elif hp == 0:
                            nc.scalar.copy(
                                out=ov_h[:, 0:2], in_=po_h[0:NLM]
                            )
                        else:
                            nc.vector.tensor_copy(
                                ov_h[:, 2:4], po_h[0:NLM]
                            )
                    else:
                        for k in range(2):
                            hd = 2 * hp + k
                            nc.vector.scalar_tensor_tensor(
                                out=ov_h[:, hd],
                                in0=hg[0:NLM, :, hd * HC : (hd + 1) * HC],
                                scalar=dg_sb[:, hd : hd + 1],
                                in1=po_h[0:NLM, k],
                                op0=mybir.AluOpType.mult,
                                op1=mybir.AluOpType.add,
                            )
                if gi == NGRP - 1:
                    h = G // 2
                    nc.sync.dma_start(
                        out[gi * GW : gi * GW + h * NLM, :].rearrange(
                            "(s i) c -> i s c", s=h
                        ),
                        osb[:, :h],
                    )
                    nc.sync.dma_start(
                        out[gi * GW + h * NLM : (gi + 1) * GW, :].rearrange(
                            "(s i) c -> i s c", s=h
                        ),
                        osb[:, h:],
                    )
                else:
                    nc.sync.dma_start(
                        out[gi * GW : (gi + 1) * GW, :].rearrange(
                            "(s i) c -> i s c", s=G
                        ),
                        osb[:],
                    )

    nc.compile()
    return nc


def _host_prep(W, e, bias, rows, cols):
    """Precompute fp16 device constants from the small parameter tensors."""
    W = np.asarray(W, np.float32)
    e = np.asarray(e, np.float32)
    bias = np.asarray(bias, np.float32)
    rows = np.asarray(rows, np.int64)
    cols = np.asarray(cols, np.int64)

    logits = np.full((HEADS, NLM, NLM), NEG, np.float64)
    logits[:, rows, cols] = e.astype(np.float64)
    m = logits.max(axis=-1, keepdims=True)
    p = np.exp(logits - m)
    A = p / p.sum(axis=-1, keepdims=True)            # [H, N, N]
    dg = np.einsum("hii->hi", A).copy()              # [H, N]
    A_off = A.copy()
    np.einsum("hii->hi", A_off)[:] = 0.0

    # Wall: [c, (s, h, d)] -> chunked [128, 2, 512]
    wr = W.transpose(2, 1, 0, 3).reshape(CIN, 2 * HEADS * HC)   # [c, shd]
    wall = np.ascontiguousarray(
        wr.reshape(2, P, 2 * HEADS * HC).transpose(1, 0, 2)
    ).astype(np.float16)

    # graph matrices [j, head, i]: A_off^T with an all-ones row 98
    # (adds the bias staged at hgrp[98]); the diag term is applied on
    # DVE via the dgv per-partition scale vector
    gm = np.zeros((P, HEADS, P), np.float32)
    gmd = np.zeros((P, HEADS, P), np.float32)
    idx = np.arange(NLM)
    for h in range(HEADS):
        gm[:NLM, h, :NLM] = A_off[h].T
        gm[NLM, h, :NLM] = 1.0
        gmd[idx, h, idx] = dg[h]

    # gm | gmd | dgv packed into one const blob
    gblob = np.zeros((P, 2 * HEADS * P + HEADS), np.float32)
    gblob[:, : HEADS * P] = gm.reshape(P, HEADS * P)
    gblob[:, HEADS * P : 2 * HEADS * P] = gmd.reshape(P, HEADS * P)
    gblob[:NLM, 2 * HEADS * P :] = dg.T
    gblob = np.ascontiguousarray(gblob).astype(np.float16)

    # hgrp rows 98..127: row 98 carries bias at the h1 column block
    brow = np.zeros((30, G, HD), np.float32)
    brow[0, :, 256:512] = bias.reshape(HEADS * HC)
    brow = np.ascontiguousarray(brow.reshape(30, G * HD)).astype(np.float16)

    return {"wall": wall, "gblob": gblob, "brow": brow}


def kernel(x, W, e, bias, rows, cols):
    from concourse.bass_utils import run_bass_kernel_spmd

    if "nc" not in _CACHE:
        _CACHE["nc"] = _build_nc()
    nc = _CACHE["nc"]

    consts = _host_prep(W, e, bias, rows, cols)
    x = np.ascontiguousarray(np.asarray(x, np.float32)).reshape(
        B * NLM, CIN
    )

    in_maps = []
    for ci in range(NCORES):
        sh = x[ci * NS * NLM : (ci + 1) * NS * NLM]          # [12544, 256]
        xT = np.zeros((CIN, WPAD), np.float16)
        xT[:, : NS * NLM] = sh.T
        in_maps.append({"xt": xT, **consts})

    res = run_bass_kernel_spmd(
        nc,
        in_maps,
        core_ids=list(range(NCORES)),
        trace=bool(int(os.environ.get("KERNEL_TRACE", "0"))),
    )
    _CACHE["last_results"] = res

    out = np.concatenate(
        [
            r["out"].astype(np.float32).reshape(NS, NLM, HEADS * HC)
            for r in res.results
        ],
        axis=0,
    )
    return out
